# revision 7
# baseline (speedup 1.0000x reference)
"""Multi-head causal attention (B=2,T=2048,C=1024,H=16,Dh=64) on 8 trn2 cores.

Sharding: tensor-parallel over heads - core c owns heads (2c, 2c+1).
Per core: QKV projections for its 128 q/k/v columns, causal attention for
its 2 heads x 2 batches, all-to-all reshard (heads->tokens), full output
projection for its 512 tokens. Host concatenates and adds the bias.

Differences vs the v1 kernel (196us):
- att@v runs "query-major": out[q, f] with wei as the stationary operand,
  so the PE contracts over the full 128-key dimension (2x fewer moving
  columns; matmul cost is moving-size only). A ones-column in v gives the
  softmax denominator for free; normalization is a per-partition
  tensor_scalar (no DRAM broadcast round-trip).
- v is projected directly into token-major layout (lhsT=x chunk), killing
  the per-tile PE transposes.
- The a2a payload stays token-major; the feature-major rhs the projection
  needs is produced by dma_start_transpose (xbar) on the way back in.
- The whole kernel is software-pipelined at strip granularity: each
  strip's K/Q run one strip ahead (they gate the scores), V just-in-time,
  and the next strip's scores are emitted before the current strip's av
  stage, so the Act engine's exp stream starts at ~10us and never starves.
  proj(b0) is hazard-pinned into the a2a(b1) window and a warm-keeper
  matmul chain keeps the PE p-state ramped for proj(b1).
- exp instructions cover only the causal range via strided APs; the
  causal mask multiply touches only true diagonal 128x128 blocks.
"""
import numpy as np
import ml_dtypes

import concourse.bass as bass
import concourse.mybir as mybir
import concourse.tile as tile
from concourse.bass_utils import run_bass_kernel_spmd
from concourse.vector_clock import ScopedClock

BF16 = mybir.dt.bfloat16
F32 = mybir.dt.float32
EXP = mybir.ActivationFunctionType.Exp

B, T, C = 2, 2048, 1024
H, DH = 16, 64
NCORES = 8
HPC = 128  # head-columns per core (2 heads x 64)
NI = 512   # query-strip width
NJ = 128   # key-tile width
NSTRIP = T // NI          # 4 strips per batch
NCC = C // 128            # 8 contraction chunks
SCALE = DH ** -0.5


class TileContextP(tile.TileContext):
    """This walrus build caps sync waits at 1 per instruction (2 for
    EventSemaphore). Tile can emit more. Legalize by spilling excess waits
    onto same-engine nops emitted just before the instruction, and do the
    same for the kernel-tail drain."""

    def _commit_instruction(self, inst, lazy_reg_writes: bool = True):
        si = getattr(inst, "sync_info", None)
        if si is not None and si.on_wait:
            cap = 2 if isinstance(inst, mybir.InstEventSemaphore) else 1
            if len(si.on_wait) > cap:
                waits = list(si.on_wait)
                keep, spill = waits[:cap - 1] if cap > 1 else [], waits[cap - 1:]
                spill, last = spill[:-1], spill[-1:]
                for w in spill:
                    nop = mybir.InstNoOp(
                        name=self.nc.get_next_instruction_name(),
                        engine=inst.engine,
                        sync_info=mybir.SyncInfo(on_wait=[w], on_update=[]),
                        bass_nofuse=True,
                    )
                    self._add_instruction(nop)
                si.on_wait = keep + last
        return super()._commit_instruction(inst, lazy_reg_writes)

    def _drain_and_barrier(self, tick_clock, wait_clock):
        probe = self.nc.sync.nop()
        wait_clock.add_sem_waits(
            probe.ins, ScopedClock({None: tick_clock.global_clock})
        )
        waits = list(probe.ins.sync_info.on_wait) if probe.ins.sync_info else []
        if probe.ins.sync_info:
            probe.ins.sync_info.on_wait = []
        for w in waits:
            n = self.nc.sync.nop()
            si = n.ins.sync_info
            if si is None:
                n.ins.sync_info = mybir.SyncInfo(on_wait=[w], on_update=[])
            else:
                si.on_wait = [w]
        self.nc.sync.drain()
        self.nc.all_engine_barrier()
        assert self.sems is not None
        popped = self.nc._tile_sem_poison_stack.pop()
        assert popped is self._sem_poison
        self.nc.clear_and_free_semaphores(list(self.sems.allocated().values()))
        self.nc.all_engine_barrier()


def build_nc():
    nc = bass.Bass()
    xT_h = nc.dram_tensor("xT", [B, C, T], BF16, kind="ExternalInput")
    wq_h = nc.dram_tensor("wq", [C, HPC], BF16, kind="ExternalInput")
    wk_h = nc.dram_tensor("wk", [C, HPC], BF16, kind="ExternalInput")
    wv_h = nc.dram_tensor("wv", [C, HPC], BF16, kind="ExternalInput")
    wp_h = nc.dram_tensor("wp", [C, C], BF16, kind="ExternalInput")
    mk_h = nc.dram_tensor("mask", [NJ, NJ], BF16, kind="ExternalInput")
    y_h = nc.dram_tensor("y_out", [C, NI], F32, kind="ExternalOutput")
    # token-major reshard buffers: slot j on core c carries c's head-pair
    # features for batch-b tokens [256j, 256j+256)
    a2a_in = [nc.dram_tensor(f"a2a_in{b}", [NCORES, 256, HPC], BF16)
              for b in range(B)]
    a2a_out = [nc.dram_tensor(f"a2a_out{b}", [NCORES, 256, HPC], BF16)
               for b in range(B)]

    with TileContextP(nc) as tc, \
         tc.tile_pool(name="singles", bufs=1) as singles, \
         tc.tile_pool(name="qkp", bufs=2) as qkp, \
         tc.tile_pool(name="vtp", bufs=2) as vtp, \
         tc.tile_pool(name="weip", bufs=30) as weip, \
         tc.tile_pool(name="attqp", bufs=6) as attqp, \
         tc.tile_pool(name="rp", bufs=4) as rp, \
         tc.tile_pool(name="rhsp", bufs=2) as rhsp, \
         tc.tile_pool(name="ydr", bufs=4) as ydr, \
         tc.tile_pool(name="scops", bufs=2, space="PSUM") as scops, \
         tc.tile_pool(name="avps", bufs=2, space="PSUM") as avps, \
         tc.tile_pool(name="bigps", bufs=2, space="PSUM") as bigps:

        # ---- weights (k first: it gates the first matmul group). Loads issue
        # from the Pool queue (SWDGE): Act stays clear for the exp stream and
        # SP for the x ranges.
        wk = singles.tile([128, NCC, HPC], BF16)
        wq = singles.tile([128, NCC, HPC], BF16)
        wv = singles.tile([128, NCC, HPC], BF16)
        for w_t, w_hh in ((wk, wk_h), (wq, wq_h), (wv, wv_h)):
            nc.gpsimd.dma_start(out=w_t, in_=w_hh.rearrange("(n p) m -> p n m", p=128))
        mask = singles.tile([128, NJ], BF16)
        nc.gpsimd.dma_start(out=mask, in_=mk_h[:, :])
        wp = singles.tile([128, NCC, C], BF16)

        # ---- x loads, by 256-token ranges so early strips unblock fast
        xts = []
        for b in range(B):
            xt = singles.tile([128, NCC, T], BF16, name=f"xt{b}")
            xsrc = xT_h[b].rearrange("(n p) t -> p n t", p=128)
            for tr in range(8):
                nc.sync.dma_start(out=xt[:, :, tr * 256:(tr + 1) * 256],
                                  in_=xsrc[:, :, tr * 256:(tr + 1) * 256])
            xts.append(xt)
            if b == 0:
                nc.sync.dma_start(out=wp,
                                  in_=wp_h.rearrange("(n p) m -> p n m", p=128))

        qts, kts, vtoks = [], [], []
        for b in range(B):
            qt = qkp.tile([128, T], BF16, tag="qt")
            kt = qkp.tile([128, T], BF16, tag="kt")
            vtok = vtp.tile([128, T // NJ, 130], BF16, tag="vt")
            # ones columns at 64 (head 0) and 129 (head 1): free softmax sums
            nc.vector.memset(vtok.rearrange("p j (h c) -> p j h c", c=65)[:, :, :, 64], 1.0)
            qts.append(qt); kts.append(kt); vtoks.append(vtok)

        def emit_kq(b, s):
            """K, Q feature-major for tokens [512s, 512s+512) — these gate the
            score matmuls, so they are emitted ahead of V."""
            sl = slice(s * NI, (s + 1) * NI)
            for w_t, dst in ((wk, kts[b]), (wq, qts[b])):
                ps = bigps.tile([128, NI], F32, tag="big")
                for cc in range(NCC):
                    nc.tensor.matmul(ps, w_t[:, cc, :], xts[b][:, cc, sl],
                                     start=(cc == 0), stop=(cc == NCC - 1))
                nc.vector.tensor_copy(dst[:, sl], ps)

        def emit_v(b, s):
            """V token-major for tokens [512s, 512s+512) — only needed by the
            av stage, so it trails the K/Q of the same strip."""
            vdst = vtoks[b].rearrange("p j (h c) -> p j h c", c=65)
            for tt in range(4):
                jt = 4 * s + tt
                vps = bigps.tile([128, NJ], F32, tag="big")
                for cc in range(NCC):
                    nc.tensor.matmul(vps,
                                     xts[b][:, cc, jt * 128:(jt + 1) * 128],
                                     wv[:, cc, :],
                                     start=(cc == 0), stop=(cc == NCC - 1))
                nc.vector.tensor_copy(
                    vdst[:, jt, :, 0:64],
                    vps.rearrange("p (h c) -> p h c", h=2))

        def att_scores(b, s):
            njt = 4 * (s + 1)
            qt, kt = qts[b], kts[b]
            weis = []
            for jt in range(njt):
                d = jt - 4 * s
                lo = max(d, 0) * 128
                j0 = jt * NJ
                sco = scops.tile([128, 2 * NI], F32, tag="sco")
                for h in range(2):
                    nc.tensor.matmul(
                        sco[:, h * NI + lo:(h + 1) * NI],
                        kt[h * 64:(h + 1) * 64, j0:j0 + NJ],
                        qt[h * 64:(h + 1) * 64, s * NI + lo:(s + 1) * NI],
                        start=True, stop=True,
                    )
                wei = weip.tile([128, 2 * NI], BF16)
                if lo:
                    w2 = wei.rearrange("p (h q) -> p h q", h=2)
                    s2 = sco.rearrange("p (h q) -> p h q", h=2)
                    nc.scalar.activation(w2[:, :, lo:], s2[:, :, lo:], EXP,
                                         scale=SCALE)
                else:
                    nc.scalar.activation(wei, sco, EXP, scale=SCALE)
                if 0 <= d < 4:
                    for h in range(2):
                        blk = slice(h * NI + d * 128, h * NI + (d + 1) * 128)
                        nc.vector.tensor_mul(wei[:, blk], wei[:, blk], mask)
                weis.append(wei)
            return weis

        def att_avs(b, s, weis):
            vtok = vtoks[b]
            att_q = attqp.tile([128, 4, NJ], BF16)
            # av: one accumulation group per (qc, h), each in its own psum
            # tile -- start=True zeroes a whole 2KB psum bank, so groups must
            # never interleave within a bank; the ring hazard serializes reuse
            for qc in range(4):
                e = 4 * s + qc
                for h in range(2):
                    avt = avps.tile([128, 65], F32, tag="av")
                    for jt in range(e + 1):
                        nc.tensor.matmul(
                            avt,
                            weis[jt][:, h * NI + qc * 128:h * NI + (qc + 1) * 128],
                            vtok[:, jt, h * 65:(h + 1) * 65],
                            start=(jt == 0), stop=(jt == e),
                        )
                    r = rp.tile([128, 1], F32)
                    nc.vector.reciprocal(r, avt[:, 64:65])
                    nc.vector.tensor_scalar_mul(
                        att_q[:, qc, h * 64:(h + 1) * 64], avt[:, 0:64], r)
            # stage the strip's 512 tokens (token-major) for the reshard.
            # MUST be on the Pool queue: the collective is Pool too, so engine
            # program order guarantees every staging lands before it fires.
            dst = a2a_in[b].rearrange("j t f -> (j t) f")[s * NI:(s + 1) * NI] \
                .rearrange("(q p) f -> p q f", p=128)
            nc.gpsimd.dma_start(out=dst, in_=att_q)

        def emit_proj(b, tail):
            # one xbar transpose brings back all 8 slots: [2048, 128] -> [128, 2048]
            rhs = rhsp.tile([128, NCORES * 256], BF16, tag="prhs")
            nc.sync.dma_start_transpose(
                rhs, a2a_out[b].rearrange("j t f -> (j t) f"))
            for nt in range(8):
                # b0's proj psum comes from the av ring: the write-after-read
                # hazard on batch-1's last normalize keeps the scheduler from
                # hoisting these matmuls into attention, where their a2a-gated
                # rhs would head-of-line-block the PE stream.
                if tail:
                    ps = bigps.tile([128, 256], F32, tag="big")
                else:
                    ps = avps.tile([128, 256], F32, tag="av")
                for j in range(NCORES):
                    nc.tensor.matmul(ps, wp[:, j, nt * 128:(nt + 1) * 128],
                                     rhs[:, j * 256:(j + 1) * 256],
                                     start=(j == 0), stop=(j == NCORES - 1))
                yo = ydr.tile([128, 256], F32)
                if tail:
                    nc.scalar.copy(yo, ps)
                else:
                    nc.vector.tensor_copy(yo, ps)
                nc.sync.dma_start(
                    out=y_h[nt * 128:(nt + 1) * 128, b * 256:(b + 1) * 256],
                    in_=yo)

        # ---- batch 0 first, with one-strip-lookahead QKV: each strip's QKV
        # burst (PE-only) lands inside the previous strip's act-bound exp
        # stretch, so neither engine starves. Batch 1's first two QKV strips
        # ride under batch 0's strip-3 exp tail.
        emit_kq(0, 0)
        pending = att_scores(0, 0)
        for s in range(NSTRIP):
            if s < 3:
                emit_kq(0, s + 1)
            else:
                emit_kq(1, 0)
            emit_v(0, s)
            nxt = att_scores(0, s + 1) if s < 3 else att_scores(1, 0)
            att_avs(0, s, pending)
            pending = nxt
        nc.gpsimd.collective_compute(
            "AllToAll", mybir.AluOpType.bypass,
            replica_groups=[list(range(NCORES))],
            ins=[a2a_in[0][:, :, :].opt()],
            outs=[a2a_out[0][:, :, :].opt()],
        )
        for s in range(NSTRIP):
            if s < 3:
                emit_kq(1, s + 1)
            emit_v(1, s)
            nxt = att_scores(1, s + 1) if s < 3 else None
            att_avs(1, s, pending)
            pending = nxt
        nc.gpsimd.collective_compute(
            "AllToAll", mybir.AluOpType.bypass,
            replica_groups=[list(range(NCORES))],
            ins=[a2a_in[1][:, :, :].opt()],
            outs=[a2a_out[1][:, :, :].opt()],
        )
        # proj(b0) fills the PE while a2a(b1) runs (emitted after the
        # collective so the scheduler cannot hoist it ahead of b1 attention,
        # where its rhs wait would head-of-line-block the PE stream)
        emit_proj(0, tail=False)
        # warm-keeper: accumulating matmul chain on a scratch psum keeps the
        # PE p-state ramped through the rest of the a2a(b1) window; result is
        # never read. The avps ring slot hazard orders it after proj(b0).
        warm = avps.tile([128, 256], F32, tag="av")
        for i in range(250):
            nc.tensor.matmul(warm, wp[:, 0, 0:128], qts[1][:, 0:256],
                             start=(i == 0), stop=(i == 250 - 1))
        emit_proj(1, tail=True)
    return nc


_NC_CACHE = {}


def _get_nc():
    if "nc" not in _NC_CACHE:
        _NC_CACHE["nc"] = build_nc()
    return _NC_CACHE["nc"]


def _host_mask():
    jl = np.arange(NJ)[:, None]
    il = np.arange(NJ)[None, :]
    return (il >= jl).astype(ml_dtypes.bfloat16)


def kernel(x, Wk, Wq, Wv, Wp, bp):
    x = np.asarray(x)
    xT = np.ascontiguousarray(x.transpose(0, 2, 1)).astype(ml_dtypes.bfloat16)
    wpb = np.asarray(Wp).astype(ml_dtypes.bfloat16)
    mask = _host_mask()
    in_maps = []
    for c in range(NCORES):
        cs = slice(c * HPC, (c + 1) * HPC)
        in_maps.append({
            "xT": xT,
            "wq": np.ascontiguousarray(Wq[:, cs]).astype(ml_dtypes.bfloat16),
            "wk": np.ascontiguousarray(Wk[:, cs]).astype(ml_dtypes.bfloat16),
            "wv": np.ascontiguousarray(Wv[:, cs]).astype(ml_dtypes.bfloat16),
            "wp": wpb,
            "mask": mask,
        })
    res = run_bass_kernel_spmd(_get_nc(), in_maps, list(range(NCORES)))
    # core c's y_out[:, b*256:(b+1)*256] covers batch-b tokens [256c, 256c+256)
    yT = np.zeros((B, C, T), np.float32)
    for c in range(NCORES):
        yo = res.results[c]["y_out"]
        for b in range(B):
            yT[b, :, 256 * c:256 * (c + 1)] = yo[:, b * 256:(b + 1) * 256]
    y = yT.transpose(0, 2, 1) + np.asarray(bp)[None, None, :]
    return np.ascontiguousarray(y, dtype=np.float32)
